# revision 26
# baseline (speedup 1.0000x reference)
"""Trainium2 Bass kernel for a ReActNet-style binary BasicBlock.

Reference math per block (twice, with different weights):
    s   = sign(x + b_in)                      # +-1
    c   = conv3x3(s, mean|w| * sign(w))       # binarized conv, pad=1
    y   = x + ALPHA * c                       # residual
    y   = prelu(y + b_mid, a) + b_out

Key facts exploited:
  * matmul inputs are exactly +-1 -> bf16 matmuls are EXACT (integer sums
    accumulated in fp32 PSUM).
  * per-output-channel weight scale factors out:  conv(s, scale*sign(w)) =
    scale .* conv(s, sign(w)).
  * residual rides through PSUM via a bf16 matmul with diag(1/(ALPHA*scale))
    (bf16 streams 1 cycle/row on the PE; fp32 costs 4): x and p1 are held /
    evicted as bf16 (~0.4% rounding, well inside the 2e-2 gate).
        T = binconv(s) + x / as           (as = ALPHA*scale, per channel)
    then prelu(x + as*binconv + b, a) = Prelu-activation(T) with
    per-partition scale=as, bias=b, alpha=a  -- a single ScalarE op reading
    PSUM directly.  (prelu positive homogeneity: as > 0.)

Layout (q4i scheme): NCHW, channels (64) on partitions; FOUR images in
flight on the four 64x64 PE tiles, each accumulating into its OWN PSUM
bank (two tiles touching one bank serialize; diagonal row-split pairs do
overlap, ~2x):
    imgA: tile(0,0)   rows 0-63  -> psum 0-63    (aligned)
    imgB: tile(64,64) rows 64-127-> psum 64-127  (aligned)
    imgC: tile(0,64) / tile(64,0) alternating per conv (crossed)
    imgD: tile(64,0) / tile(0,64) alternating per conv (crossed)
The crossed tiles flip C/D's partition home after each conv; all engine ops
stay partition-aligned, DMA handles the final placement for free.

Spatial strips of R=16 output rows.  x is DMA'd into contiguous unpadded
staging (8.9KB descriptor runs, vs 448B for a padded layout) and restrided
to the padded-114 conv layout by the Sign / bf16-cast ops whose in/out APs
differ anyway; p2 is evicted unpadded for the same reason on the store
side.  conv1 rows [h0-1,h0+1) are carried over from the previous strip's
p1 (no halo recompute); s2 is signed in chunk-sized pieces so conv2 can
start before the whole strip is evicted.

Sharding: pure data parallel, batch 32 -> 4 images x 8 cores, weights
replicated, no collectives.
"""

import numpy as np
from contextlib import ExitStack, nullcontext

import concourse.bass as bass
import concourse.tile as tile
from concourse import mybir
from concourse import bacc
from concourse.bass_utils import run_bass_kernel_spmd
from concourse.masks import make_identity

B, C, H, W = 32, 64, 112, 112
ALPHA = 0.25
NCORES = 8
BL = B // NCORES          # images per core
WP = W + 2                # padded width
R = 16                    # output rows per strip
NSTRIPS = H // R

F32 = mybir.dt.float32
BF16 = mybir.dt.bfloat16

WVEC_NAMES = ["b11", "b12", "b13", "b21", "b22", "b23", "a1", "a2"]

SKIP_STORE = False   # timing experiment: drop output DMA
SKIP_LOAD = False    # timing experiment: drop input DMA (garbage data)


def _bcast_ap(dram_ap, reps=2):
    """Source AP replicating a DRAM tensor across partition groups."""
    return bass.AP(
        tensor=dram_ap.tensor,
        offset=dram_ap.offset,
        ap=[[0, reps]] + [list(d) for d in dram_ap.ap],
    )


def _row_chunks(lo, hi, step=4):
    r = lo
    while r < hi:
        yield r, min(step, hi - r)
        r += step


def build_program(bl=BL, loop_n=None):
    """Build the Bass program for one core processing `bl` images.

    loop_n: if set, repeat the whole main loop on-device that many times
    (timing harness only -- results identical, just recomputed).
    """
    nc = bacc.Bacc("TRN2", target_bir_lowering=False, debug=False)

    x_d = nc.dram_tensor("x", [bl, C, H, W], F32, kind="ExternalInput").ap()
    w3_d = nc.dram_tensor("w3", [C, C, 3, 3], F32, kind="ExternalInput").ap()
    wpw_d = nc.dram_tensor("w_pw", [C, C, 3, 3], F32, kind="ExternalInput").ap()
    vec_d = {
        n: nc.dram_tensor(n, [C], F32, kind="ExternalInput").ap()
        for n in WVEC_NAMES
    }
    out_d = nc.dram_tensor("out", [bl, C, H, W], F32, kind="ExternalOutput").ap()

    with tile.TileContext(nc) as tc:
        _kernel_body(tc, out_d, x_d, w3_d, wpw_d, vec_d, bl, loop_n=loop_n)

    nc.compile()
    return nc


def _prep_conv_consts(nc, const, wdram, name):
    """Per-conv constants: binarized-transposed weights, as=ALPHA*mean|w|,
    diag(1/as) for the residual matmul.  Everything replicated on both
    partition halves."""
    # natural layout [co, ci*3*3] duplicated -> per-channel scale
    wn = const.tile([128, C * 9], F32, name=f"wn_{name}")
    nc.sync.dma_start(out=wn, in_=_bcast_ap(wdram.rearrange("a b c d -> a (b c d)")))
    wabs = const.tile([128, C * 9], F32, name=f"wabs_{name}")
    asum = const.tile([128, 1], F32, name=f"asum_{name}")
    nc.scalar.activation(
        out=wabs, in_=wn, func=mybir.ActivationFunctionType.Abs, accum_out=asum
    )
    asc = const.tile([128, 1], F32, name=f"asc_{name}")  # ALPHA * mean|w|
    nc.vector.tensor_scalar_mul(asc, asum, ALPHA / (C * 9))
    inv_asc = const.tile([128, 1], F32, name=f"iasc_{name}")
    nc.vector.reciprocal(inv_asc, asc)

    # gathered+transposed weights [ci(+dup), co, tap], then binarize to bf16
    wg = const.tile([128, C, 9], F32, name=f"wg_{name}")
    src = bass.AP(
        tensor=wdram.tensor,
        offset=wdram.offset,
        ap=[[9, C], [C * 9, C], [1, 9]],
    )
    for rep in range(2):
        nc.sync.dma_start(out=wg[64 * rep : 64 * rep + 64, :, :], in_=src)
    wsign = const.tile([128, C, 9], BF16, name=f"ws_{name}")
    nc.scalar.activation(out=wsign, in_=wg, func=mybir.ActivationFunctionType.Sign)

    # residual injector: diag(1/as) bf16 (bf16 matmul = 1 cyc/row vs 4 for
    # fp32), per partition half
    ident = const.tile([128, C], BF16, name=f"id_{name}")
    make_identity(nc, ident[0:64, :])
    make_identity(nc, ident[64:128, :])
    nc.vector.tensor_scalar_mul(ident, ident, inv_asc)
    return wsign, asc, ident


LO = slice(0, 64)
HI = slice(64, 128)


def _conv_quad(nc, ps, w, ident, quads, soff_of, n, asc, bias_mid, alpha,
               nr=None):
    """One chunk of conv for four images on the four 64x64 PE tiles.

    quads: list of 4 tuples (rsl, osl, s_t, res_t, roff, out_ap):
      rsl: SBUF row half this image's data lives on (slice)
      osl: PSUM partition half this image's results land on (slice)
      s_t/res_t: sign tile / residual tile; roff: residual col offset
      out_ap: where the fused Prelu eviction writes
    Each image gets its own PSUM bank so all four tiles stream
    concurrently.  Slots: residual-injector matmul then the 9 taps.
    Eviction: out = Prelu(as*psum + bias) per image (half-width ACT).
    """
    pts = [ps.tile([128, 456], F32, tag="ps", name="pt") for _ in range(4)]
    for s in range(10):
        for (rsl, osl, s_t, res_t, roff, _), pt in zip(quads, pts):
            if s == 0:
                nc.tensor.matmul(
                    pt[osl, :n], ident[rsl, :], res_t[rsl, roff : roff + n],
                    start=True, stop=False, skip_group_check=True,
                )
            else:
                t = s - 1
                so = soff_of(rsl, t)
                nc.tensor.matmul(
                    pt[osl, :n], w[rsl, :, t], s_t[rsl, so : so + n],
                    start=False, stop=(s == 9), skip_group_check=True,
                )
    for (rsl, osl, _, _, _, out_ap), pt in zip(quads, pts):
        src = pt[osl, :n]
        if nr is not None:  # strip pad columns: write unpadded output
            src = src.rearrange("p (r c) -> p r c", c=WP)[:, :, 1 : 1 + W]
        nc.scalar.activation(
            out=out_ap, in_=src,
            func=mybir.ActivationFunctionType.Prelu,
            bias=bias_mid[osl], scale=asc[osl], alpha=alpha[osl],
        )


def _kernel_body(tc, out_d, x_d, w3_d, wpw_d, vec_d, bl, loop_n=None):
    nc = tc.nc
    ctx = ExitStack()
    with ctx:
        const = ctx.enter_context(tc.tile_pool(name="const", bufs=1))
        xpool = ctx.enter_context(tc.tile_pool(name="xpool", bufs=2))
        s1pool = ctx.enter_context(tc.tile_pool(name="s1pool", bufs=2))
        p1pool = ctx.enter_context(tc.tile_pool(name="p1pool", bufs=2))
        s2pool = ctx.enter_context(tc.tile_pool(name="s2pool", bufs=2))
        p2pool = ctx.enter_context(tc.tile_pool(name="p2pool", bufs=2))
        ps = ctx.enter_context(tc.tile_pool(name="ps", bufs=8, space="PSUM"))

        # ---- constants -------------------------------------------------
        v = {}
        for n in WVEC_NAMES:
            v[n] = const.tile([128, 1], F32, name=f"v_{n}")
            nc.sync.dma_start(out=v[n], in_=_bcast_ap(vec_d[n]))
        b31 = const.tile([128, 1], F32, name="b31")  # b13 + b21
        nc.vector.tensor_tensor(
            out=b31, in0=v["b13"], in1=v["b21"], op=mybir.AluOpType.add
        )
        b32 = const.tile([128, 1], F32, name="b32")  # b13 + b22
        nc.vector.tensor_tensor(
            out=b32, in0=v["b13"], in1=v["b22"], op=mybir.AluOpType.add
        )

        w1, as1, id1 = _prep_conv_consts(nc, const, w3_d, "c1")
        w2, as2, id2 = _prep_conv_consts(nc, const, wpw_d, "c2")

        loop_cm = tc.For_i(0, loop_n, 1) if loop_n else nullcontext()
        with loop_cm:
            _main_strips(tc, nc, out_d, x_d, bl, v, b31, b32,
                         w1, as1, id1, w2, as2, id2,
                         xpool, s1pool, p1pool, s2pool, p2pool, ps)


def _main_strips(tc, nc, out_d, x_d, bl, v, b31, b32, w1, as1, id1,
                 w2, as2, id2, xpool, s1pool, p1pool, s2pool, p2pool, ps):
    X_ROWS = R + 4     # x / s1 strip rows   [h0-2, h0+R+2)
    P_ROWS = R + 2     # p1 / s2 strip rows  [h0-1, h0+R+1)
    X_LEN = X_ROWS * WP
    P_LEN = P_ROWS * WP

    # groups of 4 images (quad) or 2 (pair, AB tiles only — sim harness)
    groups = []
    i = 0
    while i < bl:
        g = min(4, bl - i)
        assert g in (2, 4)
        groups.append(list(range(i, i + g)))
        i += g

    for imgs in groups:
        nquad = len(imgs) == 4
        prev_p1_ts = None
        for s in range(NSTRIPS):
            h0 = s * R
            c1lo, c1hi = max(h0 - 1, 0), min(h0 + R + 1, H)
            # rows computed by conv1 this strip; rows [h0-1, h0+1) are
            # carried over from the previous strip's p1 (no halo recompute)
            c1start = c1lo if s == 0 else h0 + 1
            xlo, xhi = max(c1start - 1, 0), min(c1hi + 1, H)

            def xloc(g):   # global row -> local row in x/s1 strip
                return g - (h0 - 2)

            def ploc(g):   # global row -> local row in p1/s2 strip
                return g - (h0 - 1)

            # -- load x into contiguous (unpadded) staging: big DMA runs --
            nx = xhi - xlo
            x_ts = []
            for gi in range(2 if nquad else 1):
                x_t = xpool.tile([128, X_ROWS * W], F32, tag=f"x{gi}", name="x")
                x_r = x_t.rearrange("p (r c) -> p r c", c=W)
                for j in range(2):
                    if not SKIP_LOAD:
                        nc.sync.dma_start(
                            out=x_r[64 * j : 64 * j + 64,
                                    xloc(xlo) : xloc(xhi), :],
                            in_=x_d[imgs[2 * gi + j], :, xlo:xhi, :],
                        )
                    else:
                        nc.gpsimd.memset(
                            x_r[64 * j : 64 * j + 64,
                                xloc(xlo) : xloc(xhi), :], 0.5)
                x_ts.append(x_t)

            # bf16 copy of x rows [c1start, c1hi) for the conv1 residual
            # matmul; restrides contiguous -> padded for free
            xb_ts = []
            for gi, x_t in enumerate(x_ts):
                x_r = x_t.rearrange("p (r c) -> p r c", c=W)
                xb_t = s1pool.tile([128, X_LEN + 4], BF16, tag=f"xb{gi}",
                                   name="xb")
                xb_r = xb_t[:, 2 : 2 + X_LEN].rearrange(
                    "p (r c) -> p r c", c=WP
                )
                nc.vector.tensor_copy(
                    out=xb_r[:, xloc(c1start) : xloc(c1hi), 1 : 1 + W],
                    in_=x_r[:, xloc(c1start) : xloc(c1hi), :],
                )
                nc.gpsimd.memset(xb_r[:, :, 0:1], 0.0)
                nc.gpsimd.memset(xb_r[:, :, WP - 1 : WP], 0.0)
                xb_ts.append(xb_t)

            # -- s1 = sign(x + b11), zero padding ------------------------
            s1_ts = []
            for gi, x_t in enumerate(x_ts):
                x_r = x_t.rearrange("p (r c) -> p r c", c=W)
                s1_t = s1pool.tile([128, X_LEN + 4], BF16, tag=f"s1{gi}",
                                   name="s1")
                s1_r = s1_t[:, 2 : 2 + X_LEN].rearrange(
                    "p (r c) -> p r c", c=WP
                )
                nc.scalar.activation(
                    out=s1_r[:, xloc(xlo) : xloc(xhi), 1 : 1 + W],
                    in_=x_r[:, xloc(xlo) : xloc(xhi), :],
                    func=mybir.ActivationFunctionType.Sign,
                    bias=v["b11"],
                )
                nc.gpsimd.memset(s1_r[:, :, 0:1], 0.0)
                nc.gpsimd.memset(s1_r[:, :, WP - 1 : WP], 0.0)
                nc.gpsimd.memset(s1_t[:, 0:2], 0.0)
                nc.gpsimd.memset(s1_t[:, 2 + X_LEN :], 0.0)
                if xloc(xlo) > 0:  # top image edge
                    nc.gpsimd.memset(s1_t[:, 2 : 2 + xloc(xlo) * WP], 0.0)
                if xloc(xhi) < X_ROWS:  # bottom image edge
                    nc.gpsimd.memset(
                        s1_t[:, 2 + xloc(xhi) * WP : 2 + X_LEN], 0.0
                    )
                s1_ts.append(s1_t)

            # -- conv1 + fused residual/scale/bias/prelu -----------------
            # p1_AB = [p1_A(lo); p1_B(hi)]; p1_CD = [p1_D(lo); p1_C(hi)]
            # bf16: feeds Sign + the conv2 residual matmul (1 cyc/row)
            p1_ts = [
                p1pool.tile([128, P_LEN + 4], BF16, tag=f"p1{gi}", name="p1")
                for gi in range(len(x_ts))
            ]

            # carry rows [h0-1, h0+1) of p1 from the previous strip
            if s > 0:
                for gi in range(len(x_ts)):
                    nc.vector.tensor_copy(
                        out=p1_ts[gi][:, 2 : 2 + 2 * WP],
                        in_=prev_p1_ts[gi][:, 2 + R * WP : 2 + (R + 2) * WP],
                    )

            for r0, nr in _row_chunks(c1start, c1hi):
                n = nr * WP
                soff_of = (
                    lambda rsl, t, _r0=r0: 2
                    + (xloc(_r0) + t // 3 - 1) * WP + (t % 3 - 1)
                )
                roff = 2 + xloc(r0) * WP
                o1 = slice(2 + ploc(r0) * WP, 2 + ploc(r0) * WP + n)
                quads = [
                    # imgA: aligned lo->lo
                    (LO, LO, s1_ts[0], xb_ts[0], roff, p1_ts[0][LO, o1]),
                    # imgB: aligned hi->hi
                    (HI, HI, s1_ts[0], xb_ts[0], roff, p1_ts[0][HI, o1]),
                ]
                if nquad:
                    quads += [
                        # imgC: crossed lo->hi (home flips to hi for conv2)
                        (LO, HI, s1_ts[1], xb_ts[1], roff, p1_ts[1][HI, o1]),
                        # imgD: crossed hi->lo
                        (HI, LO, s1_ts[1], xb_ts[1], roff, p1_ts[1][LO, o1]),
                    ]
                _conv_quad(nc, ps, w1, id1, quads, soff_of, n,
                           as1, v["b12"], v["a1"])

            # -- s2 = sign(p1 + b13 + b21), zero padding -----------------
            # signed in chunk-sized pieces so conv2 chunks can start as
            # soon as their input rows are evicted (no whole-strip barrier)
            s2_pieces = ([(ploc(h0 - 1), 2)] if s > 0 else []) + [
                (ploc(r0), nr) for r0, nr in _row_chunks(c1start, c1hi)
            ]
            s2_ts = []
            for gi, p1_t in enumerate(p1_ts):
                s2_t = s2pool.tile([128, P_LEN + 4], BF16, tag=f"s2{gi}",
                                   name="s2")
                s2_r = s2_t[:, 2 : 2 + P_LEN].rearrange(
                    "p (r c) -> p r c", c=WP
                )
                for pr, pn in s2_pieces:
                    nc.scalar.activation(
                        out=s2_t[:, 2 + pr * WP : 2 + (pr + pn) * WP],
                        in_=p1_t[:, 2 + pr * WP : 2 + (pr + pn) * WP],
                        func=mybir.ActivationFunctionType.Sign,
                        bias=b31,
                    )
                nc.gpsimd.memset(s2_r[:, :, 0:1], 0.0)
                nc.gpsimd.memset(s2_r[:, :, WP - 1 : WP], 0.0)
                nc.gpsimd.memset(s2_t[:, 0:2], 0.0)
                nc.gpsimd.memset(s2_t[:, 2 + P_LEN :], 0.0)
                if ploc(c1lo) > 0:
                    nc.gpsimd.memset(s2_t[:, 2 : 2 + ploc(c1lo) * WP], 0.0)
                if ploc(c1hi) < P_ROWS:
                    nc.gpsimd.memset(
                        s2_t[:, 2 + ploc(c1hi) * WP : 2 + P_LEN], 0.0
                    )
                s2_ts.append(s2_t)

            # -- conv2 + fused chain -------------------------------------
            # p2_AB = [A(lo); B(hi)]; p2_CD = [C(lo); D(hi)] (crossed back)
            # unpadded layout: evictions strip pad cols; big DMA-out runs
            p2_ts = [
                p2pool.tile([128, R * W], F32, tag=f"p2{gi}", name="p2")
                for gi in range(len(x_ts))
            ]
            p2_rs = [t.rearrange("p (r c) -> p r c", c=W) for t in p2_ts]
            for r0, nr in _row_chunks(h0, h0 + R):
                n = nr * WP
                soff_of = (
                    lambda rsl, t, _r0=r0: 2
                    + (ploc(_r0) + t // 3 - 1) * WP + (t % 3 - 1)
                )
                roff = 2 + ploc(r0) * WP
                r2 = slice(r0 - h0, r0 - h0 + nr)
                quads = [
                    (LO, LO, s2_ts[0], p1_ts[0], roff, p2_rs[0][LO, r2, :]),
                    (HI, HI, s2_ts[0], p1_ts[0], roff, p2_rs[0][HI, r2, :]),
                ]
                if nquad:
                    quads += [
                        # imgC now lives on hi; crossed hi->lo back home
                        (HI, LO, s2_ts[1], p1_ts[1], roff, p2_rs[1][LO, r2, :]),
                        # imgD on lo; crossed lo->hi
                        (LO, HI, s2_ts[1], p1_ts[1], roff, p2_rs[1][HI, r2, :]),
                    ]
                _conv_quad(nc, ps, w2, id2, quads, soff_of, n,
                           as2, b32, v["a2"], nr=nr)

                # out2 = p2 + b23, per chunk (spreads DVE work)
                for gi in range(len(x_ts)):
                    p2_r = p2_rs[gi]
                    nc.vector.tensor_scalar_add(
                        p2_r[:, r2, :], p2_r[:, r2, :], v["b23"]
                    )

            # -- store -----------------------------------------------------
            for gi in range(len(x_ts)):
                p2_r = p2_rs[gi]
                for j in range(2):
                    if not SKIP_STORE:
                        nc.scalar.dma_start(
                            out=out_d[imgs[2 * gi + j], :, h0 : h0 + R, :],
                            in_=p2_r[64 * j : 64 * j + 64, :, :],
                        )
            prev_p1_ts = p1_ts


_NC_CACHE = {}


def _get_program(bl=BL):
    if bl not in _NC_CACHE:
        _NC_CACHE[bl] = build_program(bl)
    return _NC_CACHE[bl]


def make_in_maps(inputs):
    x = np.ascontiguousarray(np.asarray(inputs["x"], dtype=np.float32))
    shared = {
        "w3": np.ascontiguousarray(np.asarray(inputs["w3"], np.float32)),
        "w_pw": np.ascontiguousarray(np.asarray(inputs["w_pw"], np.float32)),
    }
    for n in WVEC_NAMES:
        shared[n] = np.ascontiguousarray(np.asarray(inputs[n], np.float32))
    return [{"x": x[i * BL : (i + 1) * BL], **shared} for i in range(NCORES)]


def run(inputs, trace=False, **kwargs):
    nc = _get_program(BL)
    res = run_bass_kernel_spmd(
        nc, make_in_maps(inputs), core_ids=list(range(NCORES)), trace=trace,
        **kwargs,
    )
    out = np.concatenate([r["out"] for r in res.results], axis=0)
    return out, res


def kernel(**inputs):
    return run(inputs)[0]


def bench(inputs, iters=20, nc=None):
    """Steady-state wall-clock benchmark: sharded jit without donation,
    device-resident inputs, async dispatch of `iters` executions."""
    import time
    import jax
    from jax.sharding import Mesh, PartitionSpec, NamedSharding
    from jax.experimental.shard_map import shard_map
    from concourse import bass2jax as b2j

    b2j.install_neuronx_cc_hook()
    if nc is None:
        nc = _get_program(BL)
    in_maps = make_in_maps(inputs)

    in_names, out_names, out_avals = [], [], []
    for alloc in nc.m.functions[0].allocations:
        if not isinstance(mybir.MemoryLocationSet, type) or not isinstance(
            alloc, mybir.MemoryLocationSet
        ):
            continue
        name = alloc.memorylocations[0].name
        if alloc.kind == "ExternalInput":
            if nc.partition_id_tensor and name == nc.partition_id_tensor.name:
                continue
            in_names.append(name)
        elif alloc.kind == "ExternalOutput":
            out_names.append(name)
            out_avals.append(
                jax.core.ShapedArray(
                    tuple(alloc.tensor_shape), mybir.dt.np(alloc.dtype)
                )
            )
    n_params = len(in_names)
    all_names = in_names + out_names
    if nc.partition_id_tensor:
        all_names = all_names + [nc.partition_id_tensor.name]

    def _body(*args):
        operands = list(args)
        if nc.partition_id_tensor:
            operands.append(b2j.partition_id_tensor())
        outs = b2j._bass_exec_p.bind(
            *operands,
            out_avals=tuple(out_avals),
            in_names=tuple(all_names),
            out_names=tuple(out_names),
            lowering_input_output_aliases=(),
            sim_require_finite=True,
            sim_require_nnan=True,
            nc=nc,
        )
        return tuple(outs)

    devices = jax.devices()[:NCORES]
    mesh = Mesh(np.asarray(devices), ("core",))
    nin = n_params + len(out_names)
    f = jax.jit(
        shard_map(
            _body,
            mesh=mesh,
            in_specs=(PartitionSpec("core"),) * nin,
            out_specs=(PartitionSpec("core"),) * len(out_names),
            check_rep=False,
        ),
        keep_unused=True,
    )
    sh = NamedSharding(mesh, PartitionSpec("core"))
    concat_in = [
        jax.device_put(np.concatenate([m[n] for m in in_maps], axis=0), sh)
        for n in in_names
    ]
    zeros = [
        jax.device_put(
            np.zeros((NCORES * a.shape[0], *a.shape[1:]), a.dtype), sh
        )
        for a in out_avals
    ]

    r = f(*concat_in, *zeros)  # warm-up / compile
    jax.block_until_ready(r)

    ts = []
    for _ in range(max(iters, 8)):
        t0 = time.perf_counter()
        r = f(*concat_in, *zeros)
        jax.block_until_ready(r)
        ts.append(time.perf_counter() - t0)
    return {"single_s": min(ts), "all": ts}


def bench_device(inputs, loops=(64, 1024), calls=10):
    """Per-iteration device time via on-device For_i repetition.  The two
    loop-count programs are dispatched in interleaved alternation so slow
    drift in dispatch overhead cancels out of the slope."""
    import time
    import jax
    from jax.sharding import Mesh, PartitionSpec, NamedSharding

    fns = {}
    for L in loops:
        nc = build_program(BL, loop_n=L)
        fns[L] = _bench_fn(inputs, nc)
    ts = {L: [] for L in loops}
    for L in loops:  # warm-up / compile
        jax.block_until_ready(fns[L]())
    for _ in range(calls):
        for L in loops:
            t0 = time.perf_counter()
            jax.block_until_ready(fns[L]())
            ts[L].append(time.perf_counter() - t0)
    res = {L: min(v) for L, v in ts.items()}
    for L in loops:
        print(f"  loop_n={L}: best single call {res[L] * 1e3:.2f} ms")
    l0, l1 = loops
    per_iter = (res[l1] - res[l0]) / (l1 - l0)
    return {"per_iter_s": per_iter, "times": res}


def _bench_fn(inputs, nc):
    """Build a zero-copy dispatch closure for `nc` (device-resident args)."""
    import jax
    from jax.sharding import Mesh, PartitionSpec, NamedSharding
    from jax.experimental.shard_map import shard_map
    from concourse import bass2jax as b2j

    b2j.install_neuronx_cc_hook()
    in_maps = make_in_maps(inputs)
    in_names, out_names, out_avals = [], [], []
    for alloc in nc.m.functions[0].allocations:
        if not isinstance(alloc, mybir.MemoryLocationSet):
            continue
        name = alloc.memorylocations[0].name
        if alloc.kind == "ExternalInput":
            if nc.partition_id_tensor and name == nc.partition_id_tensor.name:
                continue
            in_names.append(name)
        elif alloc.kind == "ExternalOutput":
            out_names.append(name)
            out_avals.append(
                jax.core.ShapedArray(
                    tuple(alloc.tensor_shape), mybir.dt.np(alloc.dtype)
                )
            )
    all_names = in_names + out_names
    if nc.partition_id_tensor:
        all_names = all_names + [nc.partition_id_tensor.name]

    def _body(*args):
        operands = list(args)
        if nc.partition_id_tensor:
            operands.append(b2j.partition_id_tensor())
        return tuple(
            b2j._bass_exec_p.bind(
                *operands,
                out_avals=tuple(out_avals),
                in_names=tuple(all_names),
                out_names=tuple(out_names),
                lowering_input_output_aliases=(),
                sim_require_finite=True,
                sim_require_nnan=True,
                nc=nc,
            )
        )

    devices = jax.devices()[:NCORES]
    mesh = Mesh(np.asarray(devices), ("core",))
    nin = len(in_names) + len(out_names)
    f = jax.jit(
        shard_map(
            _body, mesh=mesh,
            in_specs=(PartitionSpec("core"),) * nin,
            out_specs=(PartitionSpec("core"),) * len(out_names),
            check_rep=False,
        ),
        keep_unused=True,
    )
    sh = NamedSharding(mesh, PartitionSpec("core"))
    concat_in = [
        jax.device_put(np.concatenate([m[n] for m in in_maps], axis=0), sh)
        for n in in_names
    ]
    zeros = [
        jax.device_put(
            np.zeros((NCORES * a.shape[0], *a.shape[1:]), a.dtype), sh
        )
        for a in out_avals
    ]
    return lambda: f(*concat_in, *zeros)


if __name__ == "__main__":
    rng = np.random.default_rng(0)
    ins = {"x": rng.standard_normal((B, C, H, W)).astype(np.float32)}
    for n in ["w3", "w_pw"]:
        ins[n] = ((rng.random((C, C, 3, 3)) - 0.5) * 0.002).astype(np.float32)
    for n in WVEC_NAMES:
        ins[n] = (rng.standard_normal(C) * 0.01).astype(np.float32)
    out = kernel(**ins)
    print(out.shape, out.dtype)


# revision 30
# speedup vs baseline: 1.0028x; 1.0028x over previous
"""Trainium2 Bass kernel for a ReActNet-style binary BasicBlock.

Reference math per block (twice, with different weights):
    s   = sign(x + b_in)                      # +-1
    c   = conv3x3(s, mean|w| * sign(w))       # binarized conv, pad=1
    y   = x + ALPHA * c                       # residual
    y   = prelu(y + b_mid, a) + b_out

Key facts exploited:
  * matmul inputs are exactly +-1 -> bf16 matmuls are EXACT (integer sums
    accumulated in fp32 PSUM).
  * per-output-channel weight scale factors out:  conv(s, scale*sign(w)) =
    scale .* conv(s, sign(w)).
  * residual rides through PSUM via a bf16 matmul with diag(1/(ALPHA*scale))
    (bf16 streams 1 cycle/row on the PE; fp32 costs 4): x and p1 are held /
    evicted as bf16 (~0.4% rounding, well inside the 2e-2 gate).
        T = binconv(s) + x / as           (as = ALPHA*scale, per channel)
    then prelu(x + as*binconv + b, a) = Prelu-activation(T) with
    per-partition scale=as, bias=b, alpha=a  -- a single ScalarE op reading
    PSUM directly.  (prelu positive homogeneity: as > 0.)

Layout (q4i scheme): NCHW, channels (64) on partitions; FOUR images in
flight on the four 64x64 PE tiles, each accumulating into its OWN PSUM
bank (two tiles touching one bank serialize; diagonal row-split pairs do
overlap, ~2x):
    imgA: tile(0,0)   rows 0-63  -> psum 0-63    (aligned)
    imgB: tile(64,64) rows 64-127-> psum 64-127  (aligned)
    imgC: tile(0,64) / tile(64,0) alternating per conv (crossed)
    imgD: tile(64,0) / tile(0,64) alternating per conv (crossed)
The crossed tiles flip C/D's partition home after each conv; all engine ops
stay partition-aligned, DMA handles the final placement for free.

Spatial strips of R=16 output rows.  x is DMA'd into contiguous unpadded
staging (8.9KB descriptor runs, vs 448B for a padded layout) and restrided
to the padded-114 conv layout by the Sign / bf16-cast ops whose in/out APs
differ anyway; p2 is evicted unpadded for the same reason on the store
side.  conv1 rows [h0-1,h0+1) are carried over from the previous strip's
p1 (no halo recompute); s2 is signed in chunk-sized pieces so conv2 can
start before the whole strip is evicted.

Sharding: pure data parallel, batch 32 -> 4 images x 8 cores, weights
replicated, no collectives.
"""

import numpy as np
from contextlib import ExitStack, nullcontext

import concourse.bass as bass
import concourse.tile as tile
from concourse import mybir
from concourse import bacc
from concourse.bass_utils import run_bass_kernel_spmd
from concourse.masks import make_identity

B, C, H, W = 32, 64, 112, 112
ALPHA = 0.25
NCORES = 8
BL = B // NCORES          # images per core
WP = W + 2                # padded width
R = 16                    # output rows per strip
NSTRIPS = H // R

F32 = mybir.dt.float32
BF16 = mybir.dt.bfloat16

WVEC_NAMES = ["b11", "b12", "b13", "b21", "b22", "b23", "a1", "a2"]

SKIP_STORE = False   # timing experiment: drop output DMA
SKIP_LOAD = False    # timing experiment: drop input DMA (garbage data)
STORE_SPLIT = True   # alternate store DMA ring by strip parity
LOAD_SPLIT = False   # alternate load DMA ring by opposite parity
RESID_LAST = True    # residual matmul in slot 9 (vs slot 0)


def _bcast_ap(dram_ap, reps=2):
    """Source AP replicating a DRAM tensor across partition groups."""
    return bass.AP(
        tensor=dram_ap.tensor,
        offset=dram_ap.offset,
        ap=[[0, reps]] + [list(d) for d in dram_ap.ap],
    )


def _row_chunks(lo, hi, step=4):
    r = lo
    while r < hi:
        yield r, min(step, hi - r)
        r += step


def build_program(bl=BL, loop_n=None):
    """Build the Bass program for one core processing `bl` images.

    loop_n: if set, repeat the whole main loop on-device that many times
    (timing harness only -- results identical, just recomputed).
    """
    nc = bacc.Bacc("TRN2", target_bir_lowering=False, debug=False)

    x_d = nc.dram_tensor("x", [bl, C, H, W], F32, kind="ExternalInput").ap()
    w3_d = nc.dram_tensor("w3", [C, C, 3, 3], F32, kind="ExternalInput").ap()
    wpw_d = nc.dram_tensor("w_pw", [C, C, 3, 3], F32, kind="ExternalInput").ap()
    vec_d = {
        n: nc.dram_tensor(n, [C], F32, kind="ExternalInput").ap()
        for n in WVEC_NAMES
    }
    out_d = nc.dram_tensor("out", [bl, C, H, W], F32, kind="ExternalOutput").ap()

    with tile.TileContext(nc) as tc:
        _kernel_body(tc, out_d, x_d, w3_d, wpw_d, vec_d, bl, loop_n=loop_n)

    nc.compile()
    return nc


def _prep_conv_consts(nc, const, wdram, name):
    """Per-conv constants: binarized-transposed weights, as=ALPHA*mean|w|,
    diag(1/as) for the residual matmul.  Everything replicated on both
    partition halves."""
    # natural layout [co, ci*3*3] duplicated -> per-channel scale
    wn = const.tile([128, C * 9], F32, name=f"wn_{name}")
    nc.sync.dma_start(out=wn, in_=_bcast_ap(wdram.rearrange("a b c d -> a (b c d)")))
    wabs = const.tile([128, C * 9], F32, name=f"wabs_{name}")
    asum = const.tile([128, 1], F32, name=f"asum_{name}")
    nc.scalar.activation(
        out=wabs, in_=wn, func=mybir.ActivationFunctionType.Abs, accum_out=asum
    )
    asc = const.tile([128, 1], F32, name=f"asc_{name}")  # ALPHA * mean|w|
    nc.vector.tensor_scalar_mul(asc, asum, ALPHA / (C * 9))
    inv_asc = const.tile([128, 1], F32, name=f"iasc_{name}")
    nc.vector.reciprocal(inv_asc, asc)

    # gathered+transposed weights [ci(+dup), co, tap], then binarize to bf16
    wg = const.tile([128, C, 9], F32, name=f"wg_{name}")
    src = bass.AP(
        tensor=wdram.tensor,
        offset=wdram.offset,
        ap=[[9, C], [C * 9, C], [1, 9]],
    )
    for rep in range(2):
        nc.sync.dma_start(out=wg[64 * rep : 64 * rep + 64, :, :], in_=src)
    wsign = const.tile([128, C, 9], BF16, name=f"ws_{name}")
    nc.scalar.activation(out=wsign, in_=wg, func=mybir.ActivationFunctionType.Sign)

    # residual injector: diag(1/as) bf16 (bf16 matmul = 1 cyc/row vs 4 for
    # fp32), per partition half
    ident = const.tile([128, C], BF16, name=f"id_{name}")
    make_identity(nc, ident[0:64, :])
    make_identity(nc, ident[64:128, :])
    nc.vector.tensor_scalar_mul(ident, ident, inv_asc)
    return wsign, asc, ident


LO = slice(0, 64)
HI = slice(64, 128)


def _conv_quad(nc, ps, w, ident, quads, soff_of, n, asc, bias_mid, alpha,
               nr=None):
    """One chunk of conv for four images on the four 64x64 PE tiles.

    quads: list of 4 tuples (rsl, osl, s_t, res_t, roff, out_ap):
      rsl: SBUF row half this image's data lives on (slice)
      osl: PSUM partition half this image's results land on (slice)
      s_t/res_t: sign tile / residual tile; roff: residual col offset
      out_ap: where the fused Prelu eviction writes
    Each image gets its own PSUM bank so all four tiles stream
    concurrently.  Slots: residual-injector matmul then the 9 taps.
    Eviction: out = Prelu(as*psum + bias) per image (half-width ACT).
    """
    pts = [ps.tile([128, 456], F32, tag="ps", name="pt") for _ in range(4)]
    rslot = 9 if RESID_LAST else 0
    for s in range(10):
        for (rsl, osl, s_t, res_t, roff, _), pt in zip(quads, pts):
            if s == rslot:
                # residual slot: taps elsewhere need only s_t, not xb
                nc.tensor.matmul(
                    pt[osl, :n], ident[rsl, :], res_t[rsl, roff : roff + n],
                    start=(s == 0), stop=(s == 9), skip_group_check=True,
                )
            else:
                t = s - 1 if s > rslot else s
                so = soff_of(rsl, t)
                nc.tensor.matmul(
                    pt[osl, :n], w[rsl, :, t], s_t[rsl, so : so + n],
                    start=(s == 0), stop=(s == 9), skip_group_check=True,
                )
    for (rsl, osl, _, _, _, out_ap), pt in zip(quads, pts):
        src = pt[osl, :n]
        if nr is not None:  # strip pad columns: write unpadded output
            src = src.rearrange("p (r c) -> p r c", c=WP)[:, :, 1 : 1 + W]
        nc.scalar.activation(
            out=out_ap, in_=src,
            func=mybir.ActivationFunctionType.Prelu,
            bias=bias_mid[osl], scale=asc[osl], alpha=alpha[osl],
        )


def _kernel_body(tc, out_d, x_d, w3_d, wpw_d, vec_d, bl, loop_n=None):
    nc = tc.nc
    ctx = ExitStack()
    with ctx:
        const = ctx.enter_context(tc.tile_pool(name="const", bufs=1))
        xpool = ctx.enter_context(tc.tile_pool(name="xpool", bufs=2))
        s1pool = ctx.enter_context(tc.tile_pool(name="s1pool", bufs=2))
        p1pool = ctx.enter_context(tc.tile_pool(name="p1pool", bufs=2))
        s2pool = ctx.enter_context(tc.tile_pool(name="s2pool", bufs=2))
        p2pool = ctx.enter_context(tc.tile_pool(name="p2pool", bufs=2))
        ps = ctx.enter_context(tc.tile_pool(name="ps", bufs=8, space="PSUM"))

        # ---- constants -------------------------------------------------
        v = {}
        for n in WVEC_NAMES:
            v[n] = const.tile([128, 1], F32, name=f"v_{n}")
            nc.sync.dma_start(out=v[n], in_=_bcast_ap(vec_d[n]))
        b31 = const.tile([128, 1], F32, name="b31")  # b13 + b21
        nc.vector.tensor_tensor(
            out=b31, in0=v["b13"], in1=v["b21"], op=mybir.AluOpType.add
        )
        b32 = const.tile([128, 1], F32, name="b32")  # b13 + b22
        nc.vector.tensor_tensor(
            out=b32, in0=v["b13"], in1=v["b22"], op=mybir.AluOpType.add
        )

        w1, as1, id1 = _prep_conv_consts(nc, const, w3_d, "c1")
        w2, as2, id2 = _prep_conv_consts(nc, const, wpw_d, "c2")

        loop_cm = tc.For_i(0, loop_n, 1) if loop_n else nullcontext()
        with loop_cm:
            _main_strips(tc, nc, out_d, x_d, bl, v, b31, b32,
                         w1, as1, id1, w2, as2, id2,
                         xpool, s1pool, p1pool, s2pool, p2pool, ps)


def _main_strips(tc, nc, out_d, x_d, bl, v, b31, b32, w1, as1, id1,
                 w2, as2, id2, xpool, s1pool, p1pool, s2pool, p2pool, ps):
    X_ROWS = R + 4     # x / s1 strip rows   [h0-2, h0+R+2)
    P_ROWS = R + 2     # p1 / s2 strip rows  [h0-1, h0+R+1)
    X_LEN = X_ROWS * WP
    P_LEN = P_ROWS * WP

    # groups of 4 images (quad) or 2 (pair, AB tiles only — sim harness)
    groups = []
    i = 0
    while i < bl:
        g = min(4, bl - i)
        assert g in (2, 4)
        groups.append(list(range(i, i + g)))
        i += g

    for imgs in groups:
        nquad = len(imgs) == 4
        prev_p1_ts = None
        for s in range(NSTRIPS):
            h0 = s * R
            c1lo, c1hi = max(h0 - 1, 0), min(h0 + R + 1, H)
            # rows computed by conv1 this strip; rows [h0-1, h0+1) are
            # carried over from the previous strip's p1 (no halo recompute)
            c1start = c1lo if s == 0 else h0 + 1
            xlo, xhi = max(c1start - 1, 0), min(c1hi + 1, H)

            def xloc(g):   # global row -> local row in x/s1 strip
                return g - (h0 - 2)

            def ploc(g):   # global row -> local row in p1/s2 strip
                return g - (h0 - 1)

            # -- load x into contiguous (unpadded) staging: big DMA runs --
            nx = xhi - xlo
            x_ts = []
            for gi in range(2 if nquad else 1):
                x_t = xpool.tile([128, X_ROWS * W], F32, tag=f"x{gi}", name="x")
                x_r = x_t.rearrange("p (r c) -> p r c", c=W)
                ld_eng = nc.scalar if (LOAD_SPLIT and s % 2) else nc.sync
                for j in range(2):
                    if not SKIP_LOAD:
                        ld_eng.dma_start(
                            out=x_r[64 * j : 64 * j + 64,
                                    xloc(xlo) : xloc(xhi), :],
                            in_=x_d[imgs[2 * gi + j], :, xlo:xhi, :],
                        )
                    else:
                        nc.gpsimd.memset(
                            x_r[64 * j : 64 * j + 64,
                                xloc(xlo) : xloc(xhi), :], 0.5)
                x_ts.append(x_t)

            # bf16 copy of x rows [c1start, c1hi) for the conv1 residual
            # matmul; restrides contiguous -> padded for free
            xb_ts = []
            for gi, x_t in enumerate(x_ts):
                x_r = x_t.rearrange("p (r c) -> p r c", c=W)
                xb_t = s1pool.tile([128, X_LEN + 4], BF16, tag=f"xb{gi}",
                                   name="xb")
                xb_r = xb_t[:, 2 : 2 + X_LEN].rearrange(
                    "p (r c) -> p r c", c=WP
                )
                nc.vector.tensor_copy(
                    out=xb_r[:, xloc(c1start) : xloc(c1hi), 1 : 1 + W],
                    in_=x_r[:, xloc(c1start) : xloc(c1hi), :],
                )
                nc.gpsimd.memset(xb_r[:, :, 0:1], 0.0)
                nc.gpsimd.memset(xb_r[:, :, WP - 1 : WP], 0.0)
                xb_ts.append(xb_t)

            # -- s1 = sign(x + b11), zero padding ------------------------
            s1_ts = []
            for gi, x_t in enumerate(x_ts):
                x_r = x_t.rearrange("p (r c) -> p r c", c=W)
                s1_t = s1pool.tile([128, X_LEN + 4], BF16, tag=f"s1{gi}",
                                   name="s1")
                s1_r = s1_t[:, 2 : 2 + X_LEN].rearrange(
                    "p (r c) -> p r c", c=WP
                )
                nc.scalar.activation(
                    out=s1_r[:, xloc(xlo) : xloc(xhi), 1 : 1 + W],
                    in_=x_r[:, xloc(xlo) : xloc(xhi), :],
                    func=mybir.ActivationFunctionType.Sign,
                    bias=v["b11"],
                )
                nc.gpsimd.memset(s1_r[:, :, 0:1], 0.0)
                nc.gpsimd.memset(s1_r[:, :, WP - 1 : WP], 0.0)
                nc.gpsimd.memset(s1_t[:, 0:2], 0.0)
                nc.gpsimd.memset(s1_t[:, 2 + X_LEN :], 0.0)
                if xloc(xlo) > 0:  # top image edge
                    nc.gpsimd.memset(s1_t[:, 2 : 2 + xloc(xlo) * WP], 0.0)
                if xloc(xhi) < X_ROWS:  # bottom image edge
                    nc.gpsimd.memset(
                        s1_t[:, 2 + xloc(xhi) * WP : 2 + X_LEN], 0.0
                    )
                s1_ts.append(s1_t)

            # -- conv1 + fused residual/scale/bias/prelu -----------------
            # p1_AB = [p1_A(lo); p1_B(hi)]; p1_CD = [p1_D(lo); p1_C(hi)]
            # bf16: feeds Sign + the conv2 residual matmul (1 cyc/row)
            p1_ts = [
                p1pool.tile([128, P_LEN + 4], BF16, tag=f"p1{gi}", name="p1")
                for gi in range(len(x_ts))
            ]

            # carry rows [h0-1, h0+1) of p1 from the previous strip
            if s > 0:
                for gi in range(len(x_ts)):
                    nc.vector.tensor_copy(
                        out=p1_ts[gi][:, 2 : 2 + 2 * WP],
                        in_=prev_p1_ts[gi][:, 2 + R * WP : 2 + (R + 2) * WP],
                    )

            for r0, nr in _row_chunks(c1start, c1hi):
                n = nr * WP
                soff_of = (
                    lambda rsl, t, _r0=r0: 2
                    + (xloc(_r0) + t // 3 - 1) * WP + (t % 3 - 1)
                )
                roff = 2 + xloc(r0) * WP
                o1 = slice(2 + ploc(r0) * WP, 2 + ploc(r0) * WP + n)
                quads = [
                    # imgA: aligned lo->lo
                    (LO, LO, s1_ts[0], xb_ts[0], roff, p1_ts[0][LO, o1]),
                    # imgB: aligned hi->hi
                    (HI, HI, s1_ts[0], xb_ts[0], roff, p1_ts[0][HI, o1]),
                ]
                if nquad:
                    quads += [
                        # imgC: crossed lo->hi (home flips to hi for conv2)
                        (LO, HI, s1_ts[1], xb_ts[1], roff, p1_ts[1][HI, o1]),
                        # imgD: crossed hi->lo
                        (HI, LO, s1_ts[1], xb_ts[1], roff, p1_ts[1][LO, o1]),
                    ]
                _conv_quad(nc, ps, w1, id1, quads, soff_of, n,
                           as1, v["b12"], v["a1"])

            # -- s2 = sign(p1 + b13 + b21), zero padding -----------------
            # signed in chunk-sized pieces so conv2 chunks can start as
            # soon as their input rows are evicted (no whole-strip barrier)
            s2_pieces = ([(ploc(h0 - 1), 2)] if s > 0 else []) + [
                (ploc(r0), nr) for r0, nr in _row_chunks(c1start, c1hi)
            ]
            s2_ts = []
            for gi, p1_t in enumerate(p1_ts):
                s2_t = s2pool.tile([128, P_LEN + 4], BF16, tag=f"s2{gi}",
                                   name="s2")
                s2_r = s2_t[:, 2 : 2 + P_LEN].rearrange(
                    "p (r c) -> p r c", c=WP
                )
                for pr, pn in s2_pieces:
                    nc.scalar.activation(
                        out=s2_t[:, 2 + pr * WP : 2 + (pr + pn) * WP],
                        in_=p1_t[:, 2 + pr * WP : 2 + (pr + pn) * WP],
                        func=mybir.ActivationFunctionType.Sign,
                        bias=b31,
                    )
                nc.gpsimd.memset(s2_r[:, :, 0:1], 0.0)
                nc.gpsimd.memset(s2_r[:, :, WP - 1 : WP], 0.0)
                nc.gpsimd.memset(s2_t[:, 0:2], 0.0)
                nc.gpsimd.memset(s2_t[:, 2 + P_LEN :], 0.0)
                if ploc(c1lo) > 0:
                    nc.gpsimd.memset(s2_t[:, 2 : 2 + ploc(c1lo) * WP], 0.0)
                if ploc(c1hi) < P_ROWS:
                    nc.gpsimd.memset(
                        s2_t[:, 2 + ploc(c1hi) * WP : 2 + P_LEN], 0.0
                    )
                s2_ts.append(s2_t)

            # -- conv2 + fused chain -------------------------------------
            # p2_AB = [A(lo); B(hi)]; p2_CD = [C(lo); D(hi)] (crossed back)
            # unpadded layout: evictions strip pad cols; big DMA-out runs
            p2_ts = [
                p2pool.tile([128, R * W], F32, tag=f"p2{gi}", name="p2")
                for gi in range(len(x_ts))
            ]
            p2_rs = [t.rearrange("p (r c) -> p r c", c=W) for t in p2_ts]
            for r0, nr in _row_chunks(h0, h0 + R):
                n = nr * WP
                soff_of = (
                    lambda rsl, t, _r0=r0: 2
                    + (ploc(_r0) + t // 3 - 1) * WP + (t % 3 - 1)
                )
                roff = 2 + ploc(r0) * WP
                r2 = slice(r0 - h0, r0 - h0 + nr)
                quads = [
                    (LO, LO, s2_ts[0], p1_ts[0], roff, p2_rs[0][LO, r2, :]),
                    (HI, HI, s2_ts[0], p1_ts[0], roff, p2_rs[0][HI, r2, :]),
                ]
                if nquad:
                    quads += [
                        # imgC now lives on hi; crossed hi->lo back home
                        (HI, LO, s2_ts[1], p1_ts[1], roff, p2_rs[1][LO, r2, :]),
                        # imgD on lo; crossed lo->hi
                        (LO, HI, s2_ts[1], p1_ts[1], roff, p2_rs[1][HI, r2, :]),
                    ]
                _conv_quad(nc, ps, w2, id2, quads, soff_of, n,
                           as2, b32, v["a2"], nr=nr)

                # out2 = p2 + b23, per chunk (spreads DVE work)
                for gi in range(len(x_ts)):
                    p2_r = p2_rs[gi]
                    nc.vector.tensor_scalar_add(
                        p2_r[:, r2, :], p2_r[:, r2, :], v["b23"]
                    )

            # -- store -----------------------------------------------------
            st_eng = nc.sync if (STORE_SPLIT and s % 2 == 0) else nc.scalar
            for gi in range(len(x_ts)):
                p2_r = p2_rs[gi]
                for j in range(2):
                    if not SKIP_STORE:
                        st_eng.dma_start(
                            out=out_d[imgs[2 * gi + j], :, h0 : h0 + R, :],
                            in_=p2_r[64 * j : 64 * j + 64, :, :],
                        )
            prev_p1_ts = p1_ts


_NC_CACHE = {}


def _get_program(bl=BL):
    if bl not in _NC_CACHE:
        _NC_CACHE[bl] = build_program(bl)
    return _NC_CACHE[bl]


def make_in_maps(inputs):
    x = np.ascontiguousarray(np.asarray(inputs["x"], dtype=np.float32))
    shared = {
        "w3": np.ascontiguousarray(np.asarray(inputs["w3"], np.float32)),
        "w_pw": np.ascontiguousarray(np.asarray(inputs["w_pw"], np.float32)),
    }
    for n in WVEC_NAMES:
        shared[n] = np.ascontiguousarray(np.asarray(inputs[n], np.float32))
    return [{"x": x[i * BL : (i + 1) * BL], **shared} for i in range(NCORES)]


def run(inputs, trace=False, **kwargs):
    nc = _get_program(BL)
    res = run_bass_kernel_spmd(
        nc, make_in_maps(inputs), core_ids=list(range(NCORES)), trace=trace,
        **kwargs,
    )
    out = np.concatenate([r["out"] for r in res.results], axis=0)
    return out, res


def kernel(**inputs):
    return run(inputs)[0]


def bench(inputs, iters=20, nc=None):
    """Steady-state wall-clock benchmark: sharded jit without donation,
    device-resident inputs, async dispatch of `iters` executions."""
    import time
    import jax
    from jax.sharding import Mesh, PartitionSpec, NamedSharding
    from jax.experimental.shard_map import shard_map
    from concourse import bass2jax as b2j

    b2j.install_neuronx_cc_hook()
    if nc is None:
        nc = _get_program(BL)
    in_maps = make_in_maps(inputs)

    in_names, out_names, out_avals = [], [], []
    for alloc in nc.m.functions[0].allocations:
        if not isinstance(mybir.MemoryLocationSet, type) or not isinstance(
            alloc, mybir.MemoryLocationSet
        ):
            continue
        name = alloc.memorylocations[0].name
        if alloc.kind == "ExternalInput":
            if nc.partition_id_tensor and name == nc.partition_id_tensor.name:
                continue
            in_names.append(name)
        elif alloc.kind == "ExternalOutput":
            out_names.append(name)
            out_avals.append(
                jax.core.ShapedArray(
                    tuple(alloc.tensor_shape), mybir.dt.np(alloc.dtype)
                )
            )
    n_params = len(in_names)
    all_names = in_names + out_names
    if nc.partition_id_tensor:
        all_names = all_names + [nc.partition_id_tensor.name]

    def _body(*args):
        operands = list(args)
        if nc.partition_id_tensor:
            operands.append(b2j.partition_id_tensor())
        outs = b2j._bass_exec_p.bind(
            *operands,
            out_avals=tuple(out_avals),
            in_names=tuple(all_names),
            out_names=tuple(out_names),
            lowering_input_output_aliases=(),
            sim_require_finite=True,
            sim_require_nnan=True,
            nc=nc,
        )
        return tuple(outs)

    devices = jax.devices()[:NCORES]
    mesh = Mesh(np.asarray(devices), ("core",))
    nin = n_params + len(out_names)
    f = jax.jit(
        shard_map(
            _body,
            mesh=mesh,
            in_specs=(PartitionSpec("core"),) * nin,
            out_specs=(PartitionSpec("core"),) * len(out_names),
            check_rep=False,
        ),
        keep_unused=True,
    )
    sh = NamedSharding(mesh, PartitionSpec("core"))
    concat_in = [
        jax.device_put(np.concatenate([m[n] for m in in_maps], axis=0), sh)
        for n in in_names
    ]
    zeros = [
        jax.device_put(
            np.zeros((NCORES * a.shape[0], *a.shape[1:]), a.dtype), sh
        )
        for a in out_avals
    ]

    r = f(*concat_in, *zeros)  # warm-up / compile
    jax.block_until_ready(r)

    ts = []
    for _ in range(max(iters, 8)):
        t0 = time.perf_counter()
        r = f(*concat_in, *zeros)
        jax.block_until_ready(r)
        ts.append(time.perf_counter() - t0)
    return {"single_s": min(ts), "all": ts}


def bench_device(inputs, loops=(64, 1024), calls=10):
    """Per-iteration device time via on-device For_i repetition.  The two
    loop-count programs are dispatched in interleaved alternation so slow
    drift in dispatch overhead cancels out of the slope."""
    import time
    import jax
    from jax.sharding import Mesh, PartitionSpec, NamedSharding

    fns = {}
    for L in loops:
        nc = build_program(BL, loop_n=L)
        fns[L] = _bench_fn(inputs, nc)
    ts = {L: [] for L in loops}
    for L in loops:  # warm-up / compile
        jax.block_until_ready(fns[L]())
    for _ in range(calls):
        for L in loops:
            t0 = time.perf_counter()
            jax.block_until_ready(fns[L]())
            ts[L].append(time.perf_counter() - t0)
    res = {L: min(v) for L, v in ts.items()}
    for L in loops:
        print(f"  loop_n={L}: best single call {res[L] * 1e3:.2f} ms")
    l0, l1 = loops
    per_iter = (res[l1] - res[l0]) / (l1 - l0)
    return {"per_iter_s": per_iter, "times": res}


def _bench_fn(inputs, nc):
    """Build a zero-copy dispatch closure for `nc` (device-resident args)."""
    import jax
    from jax.sharding import Mesh, PartitionSpec, NamedSharding
    from jax.experimental.shard_map import shard_map
    from concourse import bass2jax as b2j

    b2j.install_neuronx_cc_hook()
    in_maps = make_in_maps(inputs)
    in_names, out_names, out_avals = [], [], []
    for alloc in nc.m.functions[0].allocations:
        if not isinstance(alloc, mybir.MemoryLocationSet):
            continue
        name = alloc.memorylocations[0].name
        if alloc.kind == "ExternalInput":
            if nc.partition_id_tensor and name == nc.partition_id_tensor.name:
                continue
            in_names.append(name)
        elif alloc.kind == "ExternalOutput":
            out_names.append(name)
            out_avals.append(
                jax.core.ShapedArray(
                    tuple(alloc.tensor_shape), mybir.dt.np(alloc.dtype)
                )
            )
    all_names = in_names + out_names
    if nc.partition_id_tensor:
        all_names = all_names + [nc.partition_id_tensor.name]

    def _body(*args):
        operands = list(args)
        if nc.partition_id_tensor:
            operands.append(b2j.partition_id_tensor())
        return tuple(
            b2j._bass_exec_p.bind(
                *operands,
                out_avals=tuple(out_avals),
                in_names=tuple(all_names),
                out_names=tuple(out_names),
                lowering_input_output_aliases=(),
                sim_require_finite=True,
                sim_require_nnan=True,
                nc=nc,
            )
        )

    devices = jax.devices()[:NCORES]
    mesh = Mesh(np.asarray(devices), ("core",))
    nin = len(in_names) + len(out_names)
    f = jax.jit(
        shard_map(
            _body, mesh=mesh,
            in_specs=(PartitionSpec("core"),) * nin,
            out_specs=(PartitionSpec("core"),) * len(out_names),
            check_rep=False,
        ),
        keep_unused=True,
    )
    sh = NamedSharding(mesh, PartitionSpec("core"))
    concat_in = [
        jax.device_put(np.concatenate([m[n] for m in in_maps], axis=0), sh)
        for n in in_names
    ]
    zeros = [
        jax.device_put(
            np.zeros((NCORES * a.shape[0], *a.shape[1:]), a.dtype), sh
        )
        for a in out_avals
    ]
    return lambda: f(*concat_in, *zeros)


if __name__ == "__main__":
    rng = np.random.default_rng(0)
    ins = {"x": rng.standard_normal((B, C, H, W)).astype(np.float32)}
    for n in ["w3", "w_pw"]:
        ins[n] = ((rng.random((C, C, 3, 3)) - 0.5) * 0.002).astype(np.float32)
    for n in WVEC_NAMES:
        ins[n] = (rng.standard_normal(C) * 0.01).astype(np.float32)
    out = kernel(**ins)
    print(out.shape, out.dtype)


# revision 36
# speedup vs baseline: 1.1404x; 1.1372x over previous
"""Trainium2 Bass kernel for a ReActNet-style binary BasicBlock.

Reference math per block (twice, with different weights):
    s   = sign(x + b_in)                      # +-1
    c   = conv3x3(s, mean|w| * sign(w))       # binarized conv, pad=1
    y   = x + ALPHA * c                       # residual
    y   = prelu(y + b_mid, a) + b_out

Key facts exploited:
  * matmul inputs are exactly +-1 -> bf16 matmuls are EXACT (integer sums
    accumulated in fp32 PSUM).
  * per-output-channel weight scale factors out:  conv(s, scale*sign(w)) =
    scale .* conv(s, sign(w)).
  * residual rides through PSUM via a bf16 matmul with diag(1/(ALPHA*scale))
    (bf16 streams 1 cycle/row on the PE; fp32 costs 4): x and p1 are held /
    evicted as bf16 (~0.4% rounding, well inside the 2e-2 gate).
        T = binconv(s) + x / as           (as = ALPHA*scale, per channel)
    then prelu(x + as*binconv + b, a) = Prelu-activation(T) with
    per-partition scale=as, bias=b, alpha=a  -- a single ScalarE op reading
    PSUM directly.  (prelu positive homogeneity: as > 0.)

Layout (q4i scheme): NCHW, channels (64) on partitions; FOUR images in
flight on the four 64x64 PE tiles, each accumulating into its OWN PSUM
bank (two tiles touching one bank serialize; diagonal row-split pairs do
overlap, ~2x):
    imgA: tile(0,0)   rows 0-63  -> psum 0-63    (aligned)
    imgB: tile(64,64) rows 64-127-> psum 64-127  (aligned)
    imgC: tile(0,64) / tile(64,0) alternating per conv (crossed)
    imgD: tile(64,0) / tile(0,64) alternating per conv (crossed)
The crossed tiles flip C/D's partition home after each conv; all engine ops
stay partition-aligned, DMA handles the final placement for free.

Spatial strips of R=16 output rows.  x is DMA'd into contiguous unpadded
staging (8.9KB descriptor runs, vs 448B for a padded layout) and restrided
to the padded-114 conv layout by the Sign / bf16-cast ops whose in/out APs
differ anyway; p2 is evicted unpadded for the same reason on the store
side.  conv1 rows [h0-1,h0+1) are carried over from the previous strip's
p1 (no halo recompute); s2 is signed in chunk-sized pieces so conv2 can
start before the whole strip is evicted.

Sharding: pure data parallel, batch 32 -> 4 images x 8 cores, weights
replicated, no collectives.
"""

import numpy as np
from contextlib import ExitStack, nullcontext

import concourse.bass as bass
import concourse.tile as tile
from concourse import mybir
from concourse import bacc
from concourse.bass_utils import run_bass_kernel_spmd
from concourse.masks import make_identity

B, C, H, W = 32, 64, 112, 112
ALPHA = 0.25
NCORES = 8
BL = B // NCORES          # images per core
WP = W + 2                # padded width
R = 16                    # output rows per strip
NSTRIPS = H // R

F32 = mybir.dt.float32
BF16 = mybir.dt.bfloat16

WVEC_NAMES = ["b11", "b12", "b13", "b21", "b22", "b23", "a1", "a2"]

SKIP_STORE = False   # timing experiment: drop output DMA
SKIP_LOAD = False    # timing experiment: drop input DMA (garbage data)
STORE_SPLIT = True   # alternate store DMA ring by strip parity
LOAD_SPLIT = False   # alternate load DMA ring by opposite parity
RESID_LAST = True    # residual matmul in slot 9 (vs slot 0)
S2_PSUM = True       # sign s2 directly from conv1 PSUM (threshold trick)


def _bcast_ap(dram_ap, reps=2):
    """Source AP replicating a DRAM tensor across partition groups."""
    return bass.AP(
        tensor=dram_ap.tensor,
        offset=dram_ap.offset,
        ap=[[0, reps]] + [list(d) for d in dram_ap.ap],
    )


def _row_chunks(lo, hi, step=4):
    r = lo
    while r < hi:
        yield r, min(step, hi - r)
        r += step


def build_program(bl=BL, loop_n=None):
    """Build the Bass program for one core processing `bl` images.

    loop_n: if set, repeat the whole main loop on-device that many times
    (timing harness only -- results identical, just recomputed).
    """
    nc = bacc.Bacc("TRN2", target_bir_lowering=False, debug=False)

    x_d = nc.dram_tensor("x", [bl, C, H, W], F32, kind="ExternalInput").ap()
    w3_d = nc.dram_tensor("w3", [C, C, 3, 3], F32, kind="ExternalInput").ap()
    wpw_d = nc.dram_tensor("w_pw", [C, C, 3, 3], F32, kind="ExternalInput").ap()
    vec_d = {
        n: nc.dram_tensor(n, [C], F32, kind="ExternalInput").ap()
        for n in WVEC_NAMES
    }
    out_d = nc.dram_tensor("out", [bl, C, H, W], F32, kind="ExternalOutput").ap()

    with tile.TileContext(nc) as tc:
        _kernel_body(tc, out_d, x_d, w3_d, wpw_d, vec_d, bl, loop_n=loop_n)

    nc.compile()
    return nc


def _prep_conv_consts(nc, const, wdram, name):
    """Per-conv constants: binarized-transposed weights, as=ALPHA*mean|w|,
    diag(1/as) for the residual matmul.  Everything replicated on both
    partition halves."""
    # natural layout [co, ci*3*3] duplicated -> per-channel scale
    wn = const.tile([128, C * 9], F32, name=f"wn_{name}")
    nc.sync.dma_start(out=wn, in_=_bcast_ap(wdram.rearrange("a b c d -> a (b c d)")))
    wabs = const.tile([128, C * 9], F32, name=f"wabs_{name}")
    asum = const.tile([128, 1], F32, name=f"asum_{name}")
    nc.scalar.activation(
        out=wabs, in_=wn, func=mybir.ActivationFunctionType.Abs, accum_out=asum
    )
    asc = const.tile([128, 1], F32, name=f"asc_{name}")  # ALPHA * mean|w|
    nc.vector.tensor_scalar_mul(asc, asum, ALPHA / (C * 9))
    inv_asc = const.tile([128, 1], F32, name=f"iasc_{name}")
    nc.vector.reciprocal(inv_asc, asc)

    # gathered+transposed weights [ci(+dup), co, tap], then binarize to bf16
    wg = const.tile([128, C, 9], F32, name=f"wg_{name}")
    src = bass.AP(
        tensor=wdram.tensor,
        offset=wdram.offset,
        ap=[[9, C], [C * 9, C], [1, 9]],
    )
    for rep in range(2):
        nc.sync.dma_start(out=wg[64 * rep : 64 * rep + 64, :, :], in_=src)
    wsign = const.tile([128, C, 9], BF16, name=f"ws_{name}")
    nc.scalar.activation(out=wsign, in_=wg, func=mybir.ActivationFunctionType.Sign)

    # residual injector: diag(1/as) bf16 (bf16 matmul = 1 cyc/row vs 4 for
    # fp32), per partition half
    ident = const.tile([128, C], BF16, name=f"id_{name}")
    make_identity(nc, ident[0:64, :])
    make_identity(nc, ident[64:128, :])
    nc.vector.tensor_scalar_mul(ident, ident, inv_asc)
    return wsign, asc, ident


LO = slice(0, 64)
HI = slice(64, 128)


def _conv_quad(nc, ps, w, ident, quads, soff_of, n, asc, bias_mid, alpha,
               nr=None, s2_outs=None, s2_bias=None):
    """One chunk of conv for four images on the four 64x64 PE tiles.

    quads: list of 4 tuples (rsl, osl, s_t, res_t, roff, out_ap):
      rsl: SBUF row half this image's data lives on (slice)
      osl: PSUM partition half this image's results land on (slice)
      s_t/res_t: sign tile / residual tile; roff: residual col offset
      out_ap: where the fused Prelu eviction writes
    Each image gets its own PSUM bank so all four tiles stream
    concurrently.  Slots: residual-injector matmul then the 9 taps.
    Eviction: out = Prelu(as*psum + bias) per image (half-width ACT).
    """
    pts = [ps.tile([128, 456], F32, tag="ps", name="pt") for _ in range(4)]
    rslot = 9 if RESID_LAST else 0
    for s in range(10):
        for (rsl, osl, s_t, res_t, roff, _), pt in zip(quads, pts):
            if s == rslot:
                # residual slot: taps elsewhere need only s_t, not xb
                nc.tensor.matmul(
                    pt[osl, :n], ident[rsl, :], res_t[rsl, roff : roff + n],
                    start=(s == 0), stop=(s == 9), skip_group_check=True,
                )
            else:
                t = s - 1 if s > rslot else s
                so = soff_of(rsl, t)
                nc.tensor.matmul(
                    pt[osl, :n], w[rsl, :, t], s_t[rsl, so : so + n],
                    start=(s == 0), stop=(s == 9), skip_group_check=True,
                )
    for qi, ((rsl, osl, _, _, _, out_ap), pt) in enumerate(zip(quads, pts)):
        if s2_outs is not None:
            # s2 straight from PSUM (in parallel with the Prelu eviction):
            # sign(prelu(u,a)+b31) == sign(u - t1) since prelu is monotonic
            psrc = pt[osl, :n].rearrange(
                "p (r c) -> p r c", c=WP)[:, :, 1 : 1 + W]
            nc.scalar.activation(
                out=s2_outs[qi], in_=psrc,
                func=mybir.ActivationFunctionType.Sign,
                bias=s2_bias[osl], scale=asc[osl],
            )
        src = pt[osl, :n]
        if nr is not None:  # strip pad columns: write unpadded output
            src = src.rearrange("p (r c) -> p r c", c=WP)[:, :, 1 : 1 + W]
        nc.scalar.activation(
            out=out_ap, in_=src,
            func=mybir.ActivationFunctionType.Prelu,
            bias=bias_mid[osl], scale=asc[osl], alpha=alpha[osl],
        )


def _kernel_body(tc, out_d, x_d, w3_d, wpw_d, vec_d, bl, loop_n=None):
    nc = tc.nc
    ctx = ExitStack()
    with ctx:
        const = ctx.enter_context(tc.tile_pool(name="const", bufs=1))
        xpool = ctx.enter_context(tc.tile_pool(name="xpool", bufs=2))
        s1pool = ctx.enter_context(tc.tile_pool(name="s1pool", bufs=2))
        p1pool = ctx.enter_context(tc.tile_pool(name="p1pool", bufs=2))
        s2pool = ctx.enter_context(tc.tile_pool(name="s2pool", bufs=2))
        p2pool = ctx.enter_context(tc.tile_pool(name="p2pool", bufs=2))
        ps = ctx.enter_context(tc.tile_pool(name="ps", bufs=8, space="PSUM"))

        # ---- constants -------------------------------------------------
        v = {}
        for n in WVEC_NAMES:
            v[n] = const.tile([128, 1], F32, name=f"v_{n}")
            nc.sync.dma_start(out=v[n], in_=_bcast_ap(vec_d[n]))
        b31 = const.tile([128, 1], F32, name="b31")  # b13 + b21
        nc.vector.tensor_tensor(
            out=b31, in0=v["b13"], in1=v["b21"], op=mybir.AluOpType.add
        )
        b32 = const.tile([128, 1], F32, name="b32")  # b13 + b22
        nc.vector.tensor_tensor(
            out=b32, in0=v["b13"], in1=v["b22"], op=mybir.AluOpType.add
        )
        # s2 = sign(prelu(u,a1)+b31) == sign(u - t1), t1 = min(-b31,-b31/a1)
        # (prelu is monotonic; u = as1*psum + b12) -> bias bs2 = b12 - t1
        inv_a1 = const.tile([128, 1], F32, name="inv_a1")
        nc.vector.reciprocal(inv_a1, v["a1"])
        nb31 = const.tile([128, 1], F32, name="nb31")
        nc.vector.tensor_scalar_mul(nb31, b31, -1.0)
        nb31a = const.tile([128, 1], F32, name="nb31a")
        nc.vector.tensor_tensor(
            out=nb31a, in0=nb31, in1=inv_a1, op=mybir.AluOpType.mult
        )
        t1 = const.tile([128, 1], F32, name="t1")
        nc.vector.tensor_tensor(
            out=t1, in0=nb31, in1=nb31a, op=mybir.AluOpType.min
        )
        bs2 = const.tile([128, 1], F32, name="bs2")
        nc.vector.tensor_tensor(
            out=bs2, in0=v["b12"], in1=t1, op=mybir.AluOpType.subtract
        )

        w1, as1, id1 = _prep_conv_consts(nc, const, w3_d, "c1")
        w2, as2, id2 = _prep_conv_consts(nc, const, wpw_d, "c2")

        loop_cm = tc.For_i(0, loop_n, 1) if loop_n else nullcontext()
        with loop_cm:
            _main_strips(tc, nc, out_d, x_d, bl, v, b31, b32, bs2,
                         w1, as1, id1, w2, as2, id2,
                         xpool, s1pool, p1pool, s2pool, p2pool, ps)


def _main_strips(tc, nc, out_d, x_d, bl, v, b31, b32, bs2, w1, as1, id1,
                 w2, as2, id2, xpool, s1pool, p1pool, s2pool, p2pool, ps):
    X_ROWS = R + 4     # x / s1 strip rows   [h0-2, h0+R+2)
    P_ROWS = R + 2     # p1 / s2 strip rows  [h0-1, h0+R+1)
    X_LEN = X_ROWS * WP
    P_LEN = P_ROWS * WP

    # groups of 4 images (quad) or 2 (pair, AB tiles only — sim harness)
    groups = []
    i = 0
    while i < bl:
        g = min(4, bl - i)
        assert g in (2, 4)
        groups.append(list(range(i, i + g)))
        i += g

    for imgs in groups:
        nquad = len(imgs) == 4
        prev_p1_ts = None
        for s in range(NSTRIPS):
            h0 = s * R
            c1lo, c1hi = max(h0 - 1, 0), min(h0 + R + 1, H)
            # rows computed by conv1 this strip; rows [h0-1, h0+1) are
            # carried over from the previous strip's p1 (no halo recompute)
            c1start = c1lo if s == 0 else h0 + 1
            xlo, xhi = max(c1start - 1, 0), min(c1hi + 1, H)

            def xloc(g):   # global row -> local row in x/s1 strip
                return g - (h0 - 2)

            def ploc(g):   # global row -> local row in p1/s2 strip
                return g - (h0 - 1)

            # -- load x into contiguous (unpadded) staging: big DMA runs --
            nx = xhi - xlo
            x_ts = []
            for gi in range(2 if nquad else 1):
                x_t = xpool.tile([128, X_ROWS * W], F32, tag=f"x{gi}", name="x")
                x_r = x_t.rearrange("p (r c) -> p r c", c=W)
                ld_eng = nc.scalar if (LOAD_SPLIT and s % 2) else nc.sync
                for j in range(2):
                    if not SKIP_LOAD:
                        ld_eng.dma_start(
                            out=x_r[64 * j : 64 * j + 64,
                                    xloc(xlo) : xloc(xhi), :],
                            in_=x_d[imgs[2 * gi + j], :, xlo:xhi, :],
                        )
                    else:
                        nc.gpsimd.memset(
                            x_r[64 * j : 64 * j + 64,
                                xloc(xlo) : xloc(xhi), :], 0.5)
                x_ts.append(x_t)

            # bf16 copy of x rows [c1start, c1hi) for the conv1 residual
            # matmul; restrides contiguous -> padded for free
            xb_ts = []
            for gi, x_t in enumerate(x_ts):
                x_r = x_t.rearrange("p (r c) -> p r c", c=W)
                xb_t = s1pool.tile([128, X_LEN + 4], BF16, tag=f"xb{gi}",
                                   name="xb")
                xb_r = xb_t[:, 2 : 2 + X_LEN].rearrange(
                    "p (r c) -> p r c", c=WP
                )
                nc.vector.tensor_copy(
                    out=xb_r[:, xloc(c1start) : xloc(c1hi), 1 : 1 + W],
                    in_=x_r[:, xloc(c1start) : xloc(c1hi), :],
                )
                nc.gpsimd.memset(xb_r[:, :, 0:1], 0.0)
                nc.gpsimd.memset(xb_r[:, :, WP - 1 : WP], 0.0)
                xb_ts.append(xb_t)

            # -- s1 = sign(x + b11), zero padding ------------------------
            s1_ts = []
            for gi, x_t in enumerate(x_ts):
                x_r = x_t.rearrange("p (r c) -> p r c", c=W)
                s1_t = s1pool.tile([128, X_LEN + 4], BF16, tag=f"s1{gi}",
                                   name="s1")
                s1_r = s1_t[:, 2 : 2 + X_LEN].rearrange(
                    "p (r c) -> p r c", c=WP
                )
                nc.scalar.activation(
                    out=s1_r[:, xloc(xlo) : xloc(xhi), 1 : 1 + W],
                    in_=x_r[:, xloc(xlo) : xloc(xhi), :],
                    func=mybir.ActivationFunctionType.Sign,
                    bias=v["b11"],
                )
                nc.gpsimd.memset(s1_r[:, :, 0:1], 0.0)
                nc.gpsimd.memset(s1_r[:, :, WP - 1 : WP], 0.0)
                nc.gpsimd.memset(s1_t[:, 0:2], 0.0)
                nc.gpsimd.memset(s1_t[:, 2 + X_LEN :], 0.0)
                if xloc(xlo) > 0:  # top image edge
                    nc.gpsimd.memset(s1_t[:, 2 : 2 + xloc(xlo) * WP], 0.0)
                if xloc(xhi) < X_ROWS:  # bottom image edge
                    nc.gpsimd.memset(
                        s1_t[:, 2 + xloc(xhi) * WP : 2 + X_LEN], 0.0
                    )
                s1_ts.append(s1_t)

            # -- conv1 + fused residual/scale/bias/prelu -----------------
            # p1_AB = [p1_A(lo); p1_B(hi)]; p1_CD = [p1_D(lo); p1_C(hi)]
            # bf16: feeds Sign + the conv2 residual matmul (1 cyc/row)
            p1_ts = [
                p1pool.tile([128, P_LEN + 4], BF16, tag=f"p1{gi}", name="p1")
                for gi in range(len(x_ts))
            ]

            # carry rows [h0-1, h0+1) of p1 from the previous strip
            if s > 0:
                for gi in range(len(x_ts)):
                    nc.vector.tensor_copy(
                        out=p1_ts[gi][:, 2 : 2 + 2 * WP],
                        in_=prev_p1_ts[gi][:, 2 + R * WP : 2 + (R + 2) * WP],
                    )

            for r0, nr in _row_chunks(c1start, c1hi):
                n = nr * WP
                soff_of = (
                    lambda rsl, t, _r0=r0: 2
                    + (xloc(_r0) + t // 3 - 1) * WP + (t % 3 - 1)
                )
                roff = 2 + xloc(r0) * WP
                o1 = slice(2 + ploc(r0) * WP, 2 + ploc(r0) * WP + n)
                quads = [
                    # imgA: aligned lo->lo
                    (LO, LO, s1_ts[0], xb_ts[0], roff, p1_ts[0][LO, o1]),
                    # imgB: aligned hi->hi
                    (HI, HI, s1_ts[0], xb_ts[0], roff, p1_ts[0][HI, o1]),
                ]
                if nquad:
                    quads += [
                        # imgC: crossed lo->hi (home flips to hi for conv2)
                        (LO, HI, s1_ts[1], xb_ts[1], roff, p1_ts[1][HI, o1]),
                        # imgD: crossed hi->lo
                        (HI, LO, s1_ts[1], xb_ts[1], roff, p1_ts[1][LO, o1]),
                    ]
                _conv_quad(nc, ps, w1, id1, quads, soff_of, n,
                           as1, v["b12"], v["a1"])

            # -- s2 = sign(p1 + b13 + b21), zero padding -----------------
            # signed in chunk-sized pieces so conv2 chunks can start as
            # soon as their input rows are evicted (no whole-strip barrier)
            s2_pieces = ([(ploc(h0 - 1), 2)] if s > 0 else []) + [
                (ploc(r0), nr) for r0, nr in _row_chunks(c1start, c1hi)
            ]
            s2_ts = []
            for gi, p1_t in enumerate(p1_ts):
                s2_t = s2pool.tile([128, P_LEN + 4], BF16, tag=f"s2{gi}",
                                   name="s2")
                s2_r = s2_t[:, 2 : 2 + P_LEN].rearrange(
                    "p (r c) -> p r c", c=WP
                )
                for pr, pn in s2_pieces:
                    nc.scalar.activation(
                        out=s2_t[:, 2 + pr * WP : 2 + (pr + pn) * WP],
                        in_=p1_t[:, 2 + pr * WP : 2 + (pr + pn) * WP],
                        func=mybir.ActivationFunctionType.Sign,
                        bias=b31,
                    )
                nc.gpsimd.memset(s2_r[:, :, 0:1], 0.0)
                nc.gpsimd.memset(s2_r[:, :, WP - 1 : WP], 0.0)
                nc.gpsimd.memset(s2_t[:, 0:2], 0.0)
                nc.gpsimd.memset(s2_t[:, 2 + P_LEN :], 0.0)
                if ploc(c1lo) > 0:
                    nc.gpsimd.memset(s2_t[:, 2 : 2 + ploc(c1lo) * WP], 0.0)
                if ploc(c1hi) < P_ROWS:
                    nc.gpsimd.memset(
                        s2_t[:, 2 + ploc(c1hi) * WP : 2 + P_LEN], 0.0
                    )
                s2_ts.append(s2_t)

            # -- conv2 + fused chain -------------------------------------
            # p2_AB = [A(lo); B(hi)]; p2_CD = [C(lo); D(hi)] (crossed back)
            # unpadded layout: evictions strip pad cols; big DMA-out runs
            p2_ts = [
                p2pool.tile([128, R * W], F32, tag=f"p2{gi}", name="p2")
                for gi in range(len(x_ts))
            ]
            p2_rs = [t.rearrange("p (r c) -> p r c", c=W) for t in p2_ts]
            for r0, nr in _row_chunks(h0, h0 + R):
                n = nr * WP
                soff_of = (
                    lambda rsl, t, _r0=r0: 2
                    + (ploc(_r0) + t // 3 - 1) * WP + (t % 3 - 1)
                )
                roff = 2 + ploc(r0) * WP
                r2 = slice(r0 - h0, r0 - h0 + nr)
                quads = [
                    (LO, LO, s2_ts[0], p1_ts[0], roff, p2_rs[0][LO, r2, :]),
                    (HI, HI, s2_ts[0], p1_ts[0], roff, p2_rs[0][HI, r2, :]),
                ]
                if nquad:
                    quads += [
                        # imgC now lives on hi; crossed hi->lo back home
                        (HI, LO, s2_ts[1], p1_ts[1], roff, p2_rs[1][LO, r2, :]),
                        # imgD on lo; crossed lo->hi
                        (LO, HI, s2_ts[1], p1_ts[1], roff, p2_rs[1][HI, r2, :]),
                    ]
                _conv_quad(nc, ps, w2, id2, quads, soff_of, n,
                           as2, b32, v["a2"], nr=nr)

                # out2 = p2 + b23, per chunk (spreads DVE work)
                for gi in range(len(x_ts)):
                    p2_r = p2_rs[gi]
                    nc.vector.tensor_scalar_add(
                        p2_r[:, r2, :], p2_r[:, r2, :], v["b23"]
                    )

            # -- store -----------------------------------------------------
            st_eng = nc.sync if (STORE_SPLIT and s % 2 == 0) else nc.scalar
            for gi in range(len(x_ts)):
                p2_r = p2_rs[gi]
                for j in range(2):
                    if not SKIP_STORE:
                        st_eng.dma_start(
                            out=out_d[imgs[2 * gi + j], :, h0 : h0 + R, :],
                            in_=p2_r[64 * j : 64 * j + 64, :, :],
                        )
            prev_p1_ts = p1_ts


_NC_CACHE = {}


def _get_program(bl=BL):
    if bl not in _NC_CACHE:
        _NC_CACHE[bl] = build_program(bl)
    return _NC_CACHE[bl]


def make_in_maps(inputs):
    x = np.ascontiguousarray(np.asarray(inputs["x"], dtype=np.float32))
    shared = {
        "w3": np.ascontiguousarray(np.asarray(inputs["w3"], np.float32)),
        "w_pw": np.ascontiguousarray(np.asarray(inputs["w_pw"], np.float32)),
    }
    for n in WVEC_NAMES:
        shared[n] = np.ascontiguousarray(np.asarray(inputs[n], np.float32))
    return [{"x": x[i * BL : (i + 1) * BL], **shared} for i in range(NCORES)]


def run(inputs, trace=False, **kwargs):
    nc = _get_program(BL)
    res = run_bass_kernel_spmd(
        nc, make_in_maps(inputs), core_ids=list(range(NCORES)), trace=trace,
        **kwargs,
    )
    out = np.concatenate([r["out"] for r in res.results], axis=0)
    return out, res


def kernel(**inputs):
    return run(inputs)[0]


def bench(inputs, iters=20, nc=None):
    """Steady-state wall-clock benchmark: sharded jit without donation,
    device-resident inputs, async dispatch of `iters` executions."""
    import time
    import jax
    from jax.sharding import Mesh, PartitionSpec, NamedSharding
    from jax.experimental.shard_map import shard_map
    from concourse import bass2jax as b2j

    b2j.install_neuronx_cc_hook()
    if nc is None:
        nc = _get_program(BL)
    in_maps = make_in_maps(inputs)

    in_names, out_names, out_avals = [], [], []
    for alloc in nc.m.functions[0].allocations:
        if not isinstance(mybir.MemoryLocationSet, type) or not isinstance(
            alloc, mybir.MemoryLocationSet
        ):
            continue
        name = alloc.memorylocations[0].name
        if alloc.kind == "ExternalInput":
            if nc.partition_id_tensor and name == nc.partition_id_tensor.name:
                continue
            in_names.append(name)
        elif alloc.kind == "ExternalOutput":
            out_names.append(name)
            out_avals.append(
                jax.core.ShapedArray(
                    tuple(alloc.tensor_shape), mybir.dt.np(alloc.dtype)
                )
            )
    n_params = len(in_names)
    all_names = in_names + out_names
    if nc.partition_id_tensor:
        all_names = all_names + [nc.partition_id_tensor.name]

    def _body(*args):
        operands = list(args)
        if nc.partition_id_tensor:
            operands.append(b2j.partition_id_tensor())
        outs = b2j._bass_exec_p.bind(
            *operands,
            out_avals=tuple(out_avals),
            in_names=tuple(all_names),
            out_names=tuple(out_names),
            lowering_input_output_aliases=(),
            sim_require_finite=True,
            sim_require_nnan=True,
            nc=nc,
        )
        return tuple(outs)

    devices = jax.devices()[:NCORES]
    mesh = Mesh(np.asarray(devices), ("core",))
    nin = n_params + len(out_names)
    f = jax.jit(
        shard_map(
            _body,
            mesh=mesh,
            in_specs=(PartitionSpec("core"),) * nin,
            out_specs=(PartitionSpec("core"),) * len(out_names),
            check_rep=False,
        ),
        keep_unused=True,
    )
    sh = NamedSharding(mesh, PartitionSpec("core"))
    concat_in = [
        jax.device_put(np.concatenate([m[n] for m in in_maps], axis=0), sh)
        for n in in_names
    ]
    zeros = [
        jax.device_put(
            np.zeros((NCORES * a.shape[0], *a.shape[1:]), a.dtype), sh
        )
        for a in out_avals
    ]

    r = f(*concat_in, *zeros)  # warm-up / compile
    jax.block_until_ready(r)

    ts = []
    for _ in range(max(iters, 8)):
        t0 = time.perf_counter()
        r = f(*concat_in, *zeros)
        jax.block_until_ready(r)
        ts.append(time.perf_counter() - t0)
    return {"single_s": min(ts), "all": ts}


def bench_device(inputs, loops=(64, 1024), calls=10):
    """Per-iteration device time via on-device For_i repetition.  The two
    loop-count programs are dispatched in interleaved alternation so slow
    drift in dispatch overhead cancels out of the slope."""
    import time
    import jax
    from jax.sharding import Mesh, PartitionSpec, NamedSharding

    fns = {}
    for L in loops:
        nc = build_program(BL, loop_n=L)
        fns[L] = _bench_fn(inputs, nc)
    ts = {L: [] for L in loops}
    for L in loops:  # warm-up / compile
        jax.block_until_ready(fns[L]())
    for _ in range(calls):
        for L in loops:
            t0 = time.perf_counter()
            jax.block_until_ready(fns[L]())
            ts[L].append(time.perf_counter() - t0)
    res = {L: min(v) for L, v in ts.items()}
    for L in loops:
        print(f"  loop_n={L}: best single call {res[L] * 1e3:.2f} ms")
    l0, l1 = loops
    per_iter = (res[l1] - res[l0]) / (l1 - l0)
    return {"per_iter_s": per_iter, "times": res}


def _bench_fn(inputs, nc):
    """Build a zero-copy dispatch closure for `nc` (device-resident args)."""
    import jax
    from jax.sharding import Mesh, PartitionSpec, NamedSharding
    from jax.experimental.shard_map import shard_map
    from concourse import bass2jax as b2j

    b2j.install_neuronx_cc_hook()
    in_maps = make_in_maps(inputs)
    in_names, out_names, out_avals = [], [], []
    for alloc in nc.m.functions[0].allocations:
        if not isinstance(alloc, mybir.MemoryLocationSet):
            continue
        name = alloc.memorylocations[0].name
        if alloc.kind == "ExternalInput":
            if nc.partition_id_tensor and name == nc.partition_id_tensor.name:
                continue
            in_names.append(name)
        elif alloc.kind == "ExternalOutput":
            out_names.append(name)
            out_avals.append(
                jax.core.ShapedArray(
                    tuple(alloc.tensor_shape), mybir.dt.np(alloc.dtype)
                )
            )
    all_names = in_names + out_names
    if nc.partition_id_tensor:
        all_names = all_names + [nc.partition_id_tensor.name]

    def _body(*args):
        operands = list(args)
        if nc.partition_id_tensor:
            operands.append(b2j.partition_id_tensor())
        return tuple(
            b2j._bass_exec_p.bind(
                *operands,
                out_avals=tuple(out_avals),
                in_names=tuple(all_names),
                out_names=tuple(out_names),
                lowering_input_output_aliases=(),
                sim_require_finite=True,
                sim_require_nnan=True,
                nc=nc,
            )
        )

    devices = jax.devices()[:NCORES]
    mesh = Mesh(np.asarray(devices), ("core",))
    nin = len(in_names) + len(out_names)
    f = jax.jit(
        shard_map(
            _body, mesh=mesh,
            in_specs=(PartitionSpec("core"),) * nin,
            out_specs=(PartitionSpec("core"),) * len(out_names),
            check_rep=False,
        ),
        keep_unused=True,
    )
    sh = NamedSharding(mesh, PartitionSpec("core"))
    concat_in = [
        jax.device_put(np.concatenate([m[n] for m in in_maps], axis=0), sh)
        for n in in_names
    ]
    zeros = [
        jax.device_put(
            np.zeros((NCORES * a.shape[0], *a.shape[1:]), a.dtype), sh
        )
        for a in out_avals
    ]
    return lambda: f(*concat_in, *zeros)


if __name__ == "__main__":
    rng = np.random.default_rng(0)
    ins = {"x": rng.standard_normal((B, C, H, W)).astype(np.float32)}
    for n in ["w3", "w_pw"]:
        ins[n] = ((rng.random((C, C, 3, 3)) - 0.5) * 0.002).astype(np.float32)
    for n in WVEC_NAMES:
        ins[n] = (rng.standard_normal(C) * 0.01).astype(np.float32)
    out = kernel(**ins)
    print(out.shape, out.dtype)
